# revision 7
# baseline (speedup 1.0000x reference)
"""Gaussian resampling kernel for Trainium2 (8 NeuronCores, SPMD).

Computes, for each batch row b:
    e = cumsum(d); c = e - d/2
    w[t, s] = softmax_s(-(t - c_s)^2 / 10)   (masked s get weight 0)
    out[t, :] = sum_s w[t, s] * x[s, :]

Strategy (v3):
  - Data-parallel over batch: 2 batches per core on 8 cores.
  - Scores are built in [S, T] layout (tokens on partitions) in ONE ACT
    pass via Derivative_Erf(z) = (2/sqrt(pi)) * exp(-z^2) with
    z = (t - c)/sqrt(10); the 2/sqrt(pi) constant cancels into the
    host-computed denominator. Masked/pad centers go to -1e4 so their
    score is exactly 0.
  - The softmax denominator depends only on (d, mask), not on x, so the
    host computes it (banded, float64) and ships rcol = 1/(C*denom) as a
    tiny input. No ones-column, no on-device reciprocals.
  - Window re-chunking: tokens are assigned to 128-token chunks by
    CENTER VALUE (quarter-frame windows with greedy spill), not by
    position. The host gathers x/bias into this order. Each chunk's
    active frame band is then ~1024+2*margin frames regardless of the
    batch's valid length, cutting matmul pairs ~98->76/core and score
    columns ~12.5k->9.7k/core vs position-chunking.
  - Output is written as int8: the host folds a scale s = 124/max|x|
    into x, so out_i8 = round(psum * rcol) with round-to-nearest +
    saturation (verified on HW); host dequantizes by 1/s. Output is a
    convex combination of x rows, so |out*s| stays under 127: no
    clipping. This cuts output DMA traffic 4x vs fp32.
  - Matmuls in fp16 accumulate [128, 768] fp32 PSUM tiles (512+256
    column split across PSUM banks). PSUM->SBUF normalize (x rcol,
    emit int8) alternates between DVE and ACT to balance engine load;
    these copies are the structural wall (PSUM reads are 1x on both).
  - Frame indices come from an int16 GpSimd iota (half the bytes of
    f32); ACT consumes int16 directly (verified bit-identical). Junk
    matmuls warm the PE clock; output DMAs grouped 4 m-chunks at a
    time (last groups 2) to amortize descriptor generation and shorten
    the tail.
"""

import math
import sys
import types

import numpy as np

# ---------------------------------------------------------------------------
# Optional NTFF-profiling plumbing. The runtime image lacks
# antenv.axon_hooks; wire a stand-in so run_bass_kernel_spmd(trace=True)
# works (used by the dev harness; the plain kernel path never traces).
try:  # pragma: no cover - best effort
    import antenv.axon_hooks  # noqa: F401
except ImportError:
    try:
        _hooks_mod = types.ModuleType("antenv.axon_hooks")
        _hook_box = [None]
        _hooks_mod.set_axon_ntff_profile_hook = (
            lambda hook: _hook_box.__setitem__(0, hook)
        )
        _hooks_mod.get_axon_ntff_profile_hook = lambda: _hook_box[0]
        sys.modules["antenv.axon_hooks"] = _hooks_mod
        from trn_agent_boot.trn_boot import _ntff_profile_via_ctypes

        _hooks_mod.set_axon_ntff_profile_hook(
            _ntff_profile_via_ctypes("/opt/axon/libaxon_pjrt.so")
        )
    except Exception:
        pass

import concourse.bacc as bacc
import concourse.mybir as mybir
import concourse.tile as tile
import concourse.bass_utils as bass_utils
from concourse.tile_rust import add_dep_helper

# Avoid S3 artifact uploads from the trace path in this container.
bass_utils.upload_artifacts = lambda tmpdir: f"local:{tmpdir}"

from concourse.bass_utils import run_bass_kernel_spmd

NCORES = 8
B, S, D, T = 16, 512, 768, 4096
VARIANCE = 10.0
BPC = B // NCORES          # batches per core
P = 128                    # partitions
KC = S // P                # token chunks (4)
MC = T // P                # output frame chunks (32)
N0 = 512                   # first matmul column split (one PSUM bank)
MARGIN = 16.0              # frames; fp16 scores underflow past |t-c|~13
DENOM_WIN = 34.0           # frames; fp32 denominator support radius
ACT_PIECE = 2048           # max free-dim length of one score ACT op
OG = 4                     # m-chunks grouped per output DMA
RSV = 1.0 / math.sqrt(VARIANCE)
C_DE = 2.0 / math.sqrt(math.pi)   # Derivative_Erf(x) = C_DE * exp(-x^2)
QMAX = 124.0               # int8 quantization headroom
WIN = T // KC              # frame window per token chunk (1024)

_PROGRAMS = {}


def _compute_bands(c_masked):
    """Per token-chunk [lo, hi) active frame range (128-aligned), unioned
    over the given batches. c_masked: (n, S) float64, pad tokens nan.
    A fully-empty chunk yields None (skipped entirely)."""
    bands = []
    for k in range(KC):
        ck = c_masked[:, k * P:(k + 1) * P]
        if np.all(np.isnan(ck)):
            bands.append(None)
            continue
        lo = np.nanmin(ck) - MARGIN
        hi = np.nanmax(ck) + MARGIN
        a = max(0, int(math.floor(lo - 1)) // P * P)
        b = min(T, -(-int(math.ceil(hi)) // P) * P)
        b = max(b, a + P)
        bands.append((a, b))
    return tuple(bands)


def _norm_engine(seq_idx, g):
    """Engine for the normalize of output-group seq_idx, slot g: DVE early
    (ACT is producing scores), then alternate."""
    if seq_idx < 3:
        return "dve"
    return "act" if (seq_idx + g) % 2 == 0 else "dve"


def _build_program(bands2):
    """bands2: per batch-slot tuple of per-chunk (a, b) bands (or None)."""
    nc = bacc.Bacc("TRN2", target_bir_lowering=False, debug=False)
    f32 = mybir.dt.float32
    f16 = mybir.dt.float16
    i16 = mybir.dt.int16
    i8 = mybir.dt.int8

    MW = KC + MC           # per-batch meta columns (bias + rcol)
    xs_d = nc.dram_tensor("xs", [BPC, S, D], f16, kind="ExternalInput").ap()
    meta_d = nc.dram_tensor("meta", [P, BPC * MW], f32,
                            kind="ExternalInput").ap()
    out_d = nc.dram_tensor("out", [BPC, T, D], i8, kind="ExternalOutput").ap()

    AF = mybir.ActivationFunctionType

    # score pieces (k, t0, t1) in frame order; matmul chunk lists per m
    pieces2, mk2 = [], []
    for bands in bands2:
        pieces = []
        for k, band in enumerate(bands):
            if band is None:
                continue
            a, b = band
            t0 = a
            while t0 < b:
                t1 = min(t0 + ACT_PIECE, b)
                pieces.append((k, t0, t1))
                t0 = t1
        pieces.sort(key=lambda p: (p[1], p[0]))
        if pieces and pieces[0][2] - pieces[0][1] > 1024:
            k, t0, t1 = pieces[0]
            pieces[0:1] = [(k, t0, t0 + 512), (k, t0 + 512, t0 + 1024),
                           (k, t0 + 1024, t1)]
        pieces2.append(pieces)
        mk = []
        for m in range(MC):
            ks = [k for k, band in enumerate(bands)
                  if band and m * P < band[1] and (m + 1) * P > band[0]]
            assert ks, f"no active token chunk for m={m}"
            mk.append(ks)
        mk2.append(mk)

    with tile.TileContext(nc) as tc:
        with tc.tile_pool(name="const", bufs=1) as constp, \
             tc.tile_pool(name="sb", bufs=2) as sb, \
             tc.tile_pool(name="outp", bufs=6) as outp, \
             tc.tile_pool(name="colp", bufs=4) as colp, \
             tc.tile_pool(name="ps", bufs=4, space="PSUM") as ps:

            # Warm the ACT table set (erf_derivative: Derivative_Erf+Copy)
            # before any real work.
            warm = colp.tile([P, 1], f32, name="warm", tag="warm", bufs=1)
            nc.vector.memset(warm[:], 0.0)
            nc.scalar.activation(warm[:], warm[:], AF.Derivative_Erf)

            # Warm the PE HAM clock gate: junk matmuls while the real
            # inputs are still loading, so real matmuls run at 2.4GHz.
            junk = constp.tile([P, 512], f16)
            nc.gpsimd.memset(junk[:], 0.0)
            for _ in range(6):
                jp = ps.tile([P, 512], f32, name="jp", tag="pt")
                nc.tensor.matmul(jp[:], junk[:, 0:P], junk[:],
                                 start=True, stop=True)

            # meta (bias + rcol, both batches) in ONE DMA: packing gives
            # 1152B-per-partition descriptors instead of 16B ones, so it
            # lands ~3us earlier and unblocks the first score op.
            meta = constp.tile([P, BPC, MW], f32)
            nc.sync.dma_start(
                out=meta[:], in_=meta_d.rearrange("p (b w) -> p b w", b=BPC)
            )

            # trow (frame indices 1..T) from GpSimd iota, int16 (half the
            # bytes of f32; ACT consumes int16 directly). First piece
            # matches the first score op's range so scores start early.
            trow = constp.tile([P, T], i16)
            iota_cuts = [0, 512, 1024, 2048, 3072, 4096]
            for q0, q1 in zip(iota_cuts, iota_cuts[1:]):
                nc.gpsimd.iota(trow[:, q0:q1],
                               pattern=[[1, q1 - q0]], base=1 + q0,
                               channel_multiplier=0)

            # xs input DMAs up front on the Sync queue, before any output
            # issue can block them (the queue drains in program order).
            tiles = []
            for b in range(BPC):
                bcol = meta[:, b, 0:KC]
                rcolt = meta[:, b, KC:MW]
                xs = sb.tile([P, KC, D], f16, name="xs_t", tag="xs_t")
                xs_src = xs_d[b].rearrange("(k p) d -> p k d", p=P)
                for k in range(KC):
                    nc.sync.dma_start(
                        out=xs[:, k:k + 1, :], in_=xs_src[:, k:k + 1, :]
                    )
                tiles.append((bcol, rcolt, xs))

            # All score production first (ACT stream order), so batch 1's
            # scores don't queue behind batch 0's ACT-side normalizations.
            score_tiles = []
            for b in range(BPC):
                bcol, rcolt, xs = tiles[b]
                scores = sb.tile([P, KC, T], f16, name="scores", tag="scores")
                for k, t0, t1 in pieces2[b]:
                    nc.scalar.activation(
                        scores[:, k, t0:t1], trow[:, t0:t1], AF.Derivative_Erf,
                        bias=bcol[:, k:k + 1], scale=RSV,
                    )
                score_tiles.append(scores)

            # Output-group order: batch 0 leads while batch 1's scores are
            # still being produced, then the two batches interleave so the
            # engines see no cliff at the batch transition. Final groups
            # are split small to shorten the drain tail.
            def batch_groups():
                gs = [list(range(g * OG, (g + 1) * OG))
                      for g in range(MC // OG)]
                return gs[:-1] + [gs[-1][0:2], gs[-1][2:4]]

            g0, g1 = batch_groups(), batch_groups()
            group_seq = [(0, g0[i]) for i in range(3)]
            for i in range(5):
                group_seq.append((0, g0[3 + i]))
                group_seq.append((1, g1[i]))
            group_seq.append((0, g0[8]))
            for i in range(4):
                group_seq.append((1, g1[5 + i]))

            for seq_idx, (b, ms) in enumerate(group_seq):
                bcol, rcolt, xs = tiles[b]
                scores = score_tiles[b]
                ot = outp.tile([P, len(ms), D], i8, name="ot", tag="ot")
                for g, m in enumerate(ms):
                    ks = mk2[b][m]
                    pt = ps.tile([P, D], f32, name="pt", tag="pt")
                    for i, k in enumerate(ks):
                        lhsT = scores[:, k, m * P:(m + 1) * P]
                        st = (i == 0)
                        sp = (i == len(ks) - 1)
                        mma = nc.tensor.matmul(
                            pt[:, 0:N0], lhsT, xs[:, k, 0:N0],
                            start=st, stop=sp,
                        )
                        mmb = nc.tensor.matmul(
                            pt[:, N0:D], lhsT, xs[:, k, N0:D],
                            start=st, stop=sp,
                        )
                        add_dep_helper(mmb.ins, mma.ins,
                                       reason="keep N-pieces adjacent")
                    if _norm_engine(seq_idx, g) == "act":
                        nc.scalar.activation(
                            ot[:, g, :], pt[:], AF.Copy,
                            scale=rcolt[:, m:m + 1],
                        )
                    else:
                        nc.vector.tensor_scalar_mul(
                            ot[:, g, :], pt[:], rcolt[:, m:m + 1]
                        )
                nc.sync.dma_start(
                    out=out_d[b, ms[0] * P:(ms[-1] + 1) * P, :]
                    .rearrange("(g p) d -> p g d", p=P),
                    in_=ot[:],
                )

    nc.compile()
    return nc


def _get_program(bands):
    prog = _PROGRAMS.get(bands)
    if prog is None:
        prog = _build_program(bands)
        _PROGRAMS[bands] = prog
    return prog


def _denominators(c, mask):
    """Banded softmax denominators: den[b, t-1] = sum_s exp(-(t-c_s)^2/10)
    over valid s, float64, windowed to |t-c| <= DENOM_WIN (terms beyond
    are < 1e-50: irrelevant at fp32)."""
    den = np.zeros((B, T), dtype=np.float64)
    t = np.arange(1, T + 1, dtype=np.float64)
    for b in range(B):
        cb = c[b][mask[b]]
        lo = np.searchsorted(cb, t - DENOM_WIN)
        hi = np.searchsorted(cb, t + DENOM_WIN)
        w = int(np.max(hi - lo)) if len(cb) else 0
        if w == 0:
            continue
        idx = lo[:, None] + np.arange(w)[None, :]
        valid = idx < hi[:, None]
        idx = np.minimum(idx, len(cb) - 1)
        z = t[:, None] - cb[idx]
        terms = np.exp(-(z * z) / VARIANCE) * valid
        den[b] = terms.sum(axis=1)
    return den


def _assign_chunks(cb):
    """Assign sorted centers to KC chunks of capacity P by frame window
    (chunk k targets centers < (k+1)*WIN), greedy forward spill on
    overflow. Returns per-chunk lists of token indices into cb."""
    chunks = [[] for _ in range(KC)]
    k = 0
    for j, cv in enumerate(cb):
        while k < KC - 1 and (cv >= (k + 1) * WIN or len(chunks[k]) >= P):
            k += 1
        kk = k
        while len(chunks[kk]) >= P:
            kk += 1
        chunks[kk].append(j)
    return chunks


def _prepare(x, d, mask):
    x = np.asarray(x, dtype=np.float32)
    d64 = np.asarray(d, dtype=np.float64)
    mask = np.asarray(mask, dtype=bool)

    e = np.cumsum(d64, axis=-1)
    c = e - 0.5 * d64                      # (B, S) token centers

    # Window re-chunking: gather tokens into center-value chunks.
    scale = QMAX / max(float(np.abs(x).max()), 1e-30)
    xs_all = np.zeros((B, S, D), dtype=np.float16)
    cg = np.full((B, S), np.nan)           # gathered centers (nan = pad)
    for b in range(B):
        valid = np.nonzero(mask[b])[0]     # ascending position = ascending c
        cb = c[b][valid]
        for k, idxs in enumerate(_assign_chunks(cb)):
            if not idxs:
                continue
            src = valid[idxs]
            dst = slice(k * P, k * P + len(idxs))
            xs_all[b, dst] = (x[b, src] * scale).astype(np.float16)
            cg[b, dst] = c[b, src]

    # Sort batches by valid length into per-core slots (similar lengths
    # share a slot so the per-slot band unions stay tight).
    order = np.argsort(mask.sum(1), kind="stable")
    bands2 = tuple(
        _compute_bands(cg[order[s * NCORES:(s + 1) * NCORES]])
        for s in range(BPC)
    )

    cbias = np.where(np.isnan(cg), -1.0e4, cg)   # pad tokens: derf gives 0
    bias = (-cbias * RSV).astype(np.float32)

    den = _denominators(c, mask)           # (B, T) float64
    rcol = (1.0 / (C_DE * den)).astype(np.float32)    # (B, T)
    rcol = rcol.reshape(B, MC, P).transpose(0, 2, 1)  # (B, P, MC)

    MW = KC + MC
    in_maps = []
    for core in range(NCORES):
        idx = [order[core], order[NCORES + core]]
        meta = np.empty((P, BPC, MW), dtype=np.float32)
        for b, bi in enumerate(idx):
            meta[:, b, 0:KC] = bias[bi].reshape(KC, P).T
            meta[:, b, KC:MW] = rcol[bi]
        in_maps.append({
            "xs": np.ascontiguousarray(xs_all[idx]),
            "meta": meta.reshape(P, BPC * MW).copy(),
        })
    return in_maps, bands2, order, scale


def run(x, d, mask, frame_length, trace=False):
    assert int(frame_length) == T
    in_maps, bands2, order, scale = _prepare(x, d, mask)
    nc = _get_program(bands2)
    res = None
    for attempt in range(3):
        try:
            res = run_bass_kernel_spmd(nc, in_maps, list(range(NCORES)),
                                       trace=trace)
            break
        except Exception:
            # The first execution after a fresh compile occasionally hits a
            # transient device error; retrying succeeds.
            if attempt == 2:
                raise
    inv = np.float32(1.0 / scale)
    out = np.empty((B, T, D), dtype=np.float32)
    for core in range(NCORES):
        for s in range(BPC):
            q = res.results[core]["out"][s]
            out[order[s * NCORES + core]] = q.astype(np.float32) * inv
    return out, res


def kernel(x, d, mask, frame_length):
    out, _ = run(x, d, mask, frame_length, trace=False)
    return out


# revision 10
# speedup vs baseline: 1.1890x; 1.1890x over previous
"""Gaussian resampling kernel for Trainium2 (8 NeuronCores, SPMD).

Computes, for each batch row b:
    e = cumsum(d); c = e - d/2
    w[t, s] = softmax_s(-(t - c_s)^2 / 10)   (masked s get weight 0)
    out[t, :] = sum_s w[t, s] * x[s, :]

Strategy (v3):
  - Data-parallel over batch: 2 batches per core on 8 cores.
  - Scores are built in [S, T] layout (tokens on partitions) in ONE ACT
    pass via Derivative_Erf(z) = (2/sqrt(pi)) * exp(-z^2) with
    z = (t - c)/sqrt(10); the 2/sqrt(pi) constant cancels into the
    host-computed denominator. Masked/pad centers go to -1e4 so their
    score is exactly 0.
  - The softmax denominator depends only on (d, mask), not on x, so the
    host computes it (banded, float64) and ships rcol = 1/(C*denom) as a
    tiny input. No ones-column, no on-device reciprocals.
  - Window re-chunking: tokens are assigned to 128-token chunks by
    CENTER VALUE (quarter-frame windows with greedy spill), not by
    position. The host gathers x/bias into this order. Each chunk's
    active frame band is then ~1024+2*margin frames regardless of the
    batch's valid length, cutting matmul pairs ~98->76/core and score
    columns ~12.5k->9.7k/core vs position-chunking.
  - Output is written as int8: the host folds a scale s = 124/max|x|
    into x, so out_i8 = round(psum * rcol) with round-to-nearest +
    saturation (verified on HW); host dequantizes by 1/s. Output is a
    convex combination of x rows, so |out*s| stays under 127: no
    clipping. This cuts output DMA traffic 4x vs fp32.
  - Matmuls in fp16 accumulate [128, 768] fp32 PSUM tiles (512+256
    column split across PSUM banks). PSUM->SBUF normalize (x rcol,
    emit int8) alternates between DVE and ACT to balance engine load;
    these copies are the structural wall (PSUM reads are 1x on both).
  - Frame indices come from an int16 GpSimd iota (half the bytes of
    f32); ACT consumes int16 directly (verified bit-identical). Junk
    matmuls warm the PE clock; output DMAs grouped 4 m-chunks at a
    time (last groups 2) to amortize descriptor generation and shorten
    the tail.
"""

import math
import sys
import types

import numpy as np

# ---------------------------------------------------------------------------
# Optional NTFF-profiling plumbing. The runtime image lacks
# antenv.axon_hooks; wire a stand-in so run_bass_kernel_spmd(trace=True)
# works (used by the dev harness; the plain kernel path never traces).
try:  # pragma: no cover - best effort
    import antenv.axon_hooks  # noqa: F401
except ImportError:
    try:
        _hooks_mod = types.ModuleType("antenv.axon_hooks")
        _hook_box = [None]
        _hooks_mod.set_axon_ntff_profile_hook = (
            lambda hook: _hook_box.__setitem__(0, hook)
        )
        _hooks_mod.get_axon_ntff_profile_hook = lambda: _hook_box[0]
        sys.modules["antenv.axon_hooks"] = _hooks_mod
        from trn_agent_boot.trn_boot import _ntff_profile_via_ctypes

        _hooks_mod.set_axon_ntff_profile_hook(
            _ntff_profile_via_ctypes("/opt/axon/libaxon_pjrt.so")
        )
    except Exception:
        pass

import concourse.bacc as bacc
import concourse.mybir as mybir
import concourse.tile as tile
import concourse.bass_utils as bass_utils
from concourse.tile_rust import add_dep_helper

# Avoid S3 artifact uploads from the trace path in this container.
bass_utils.upload_artifacts = lambda tmpdir: f"local:{tmpdir}"

from concourse.bass_utils import run_bass_kernel_spmd

NCORES = 8
B, S, D, T = 16, 512, 768, 4096
VARIANCE = 10.0
BPC = B // NCORES          # batches per core
P = 128                    # partitions
KC = S // P                # token chunks (4)
MC = T // P                # output frame chunks (32)
N0 = 512                   # first matmul column split (one PSUM bank)
MARGIN = 16.0              # frames; fp16 scores underflow past |t-c|~13
DENOM_WIN = 34.0           # frames; fp32 denominator support radius
ACT_PIECE = 2048           # max free-dim length of one score ACT op
OG = 4                     # m-chunks grouped per output DMA
RSV = 1.0 / math.sqrt(VARIANCE)
C_DE = 2.0 / math.sqrt(math.pi)   # Derivative_Erf(x) = C_DE * exp(-x^2)
QMAX = 124.0               # int8 quantization headroom
WIN = T // KC              # frame window per token chunk (1024)

_PROGRAMS = {}


def _compute_bands(c_masked):
    """Per token-chunk [lo, hi) active frame range (128-aligned), unioned
    over the given batches. c_masked: (n, S) float64, pad tokens nan.
    A fully-empty chunk yields None (skipped entirely)."""
    bands = []
    for k in range(KC):
        ck = c_masked[:, k * P:(k + 1) * P]
        if np.all(np.isnan(ck)):
            bands.append(None)
            continue
        lo = np.nanmin(ck) - MARGIN
        hi = np.nanmax(ck) + MARGIN
        a = max(0, int(math.floor(lo - 1)) // P * P)
        b = min(T, -(-int(math.ceil(hi)) // P) * P)
        b = max(b, a + P)
        bands.append((a, b))
    return tuple(bands)


ACT_FIX = 352.0 / 1.2      # ScalarE per-op fixed cost, ns
ACT_RATE = 1.0 / 1.2       # ScalarE ns per column
DVE_COPY = 925.0           # measured DVE normalize cost, ns
ACT_COPY = 990.0           # measured ACT normalize cost, ns


def _build_program(bands2):
    """bands2: per batch-slot tuple of per-chunk (a, b) bands (or None)."""
    nc = bacc.Bacc("TRN2", target_bir_lowering=False, debug=False)
    f32 = mybir.dt.float32
    f16 = mybir.dt.float16
    i16 = mybir.dt.int16
    i8 = mybir.dt.int8

    MW = KC + MC           # per-batch meta columns (bias + rcol)
    xs_d = nc.dram_tensor("xs", [BPC, S, D], f16, kind="ExternalInput").ap()
    meta_d = nc.dram_tensor("meta", [P, BPC * MW], f32,
                            kind="ExternalInput").ap()
    out_d = nc.dram_tensor("out", [BPC, T, D], i8, kind="ExternalOutput").ap()

    AF = mybir.ActivationFunctionType

    # score pieces (k, t0, t1) in frame order; matmul chunk lists per m
    pieces2, mk2 = [], []
    for bands in bands2:
        pieces = []
        for k, band in enumerate(bands):
            if band is None:
                continue
            a, b = band
            t0 = a
            while t0 < b:
                t1 = min(t0 + ACT_PIECE, b)
                pieces.append((k, t0, t1))
                t0 = t1
        pieces.sort(key=lambda p: (p[1], p[0]))
        if pieces and pieces[0][2] - pieces[0][1] > 1024:
            k, t0, t1 = pieces[0]
            pieces[0:1] = [(k, t0, t0 + 512), (k, t0 + 512, t0 + 1024),
                           (k, t0 + 1024, t1)]
        pieces2.append(pieces)
        mk = []
        for m in range(MC):
            ks = [k for k, band in enumerate(bands)
                  if band and m * P < band[1] and (m + 1) * P > band[0]]
            assert ks, f"no active token chunk for m={m}"
            mk.append(ks)
        mk2.append(mk)

    with tile.TileContext(nc) as tc:
        with tc.tile_pool(name="const", bufs=1) as constp, \
             tc.tile_pool(name="sb", bufs=2) as sb, \
             tc.tile_pool(name="outp", bufs=6) as outp, \
             tc.tile_pool(name="colp", bufs=4) as colp, \
             tc.tile_pool(name="ps", bufs=4, space="PSUM") as ps:

            # Warm the ACT table set (erf_derivative: Derivative_Erf+Copy)
            # before any real work.
            warm = colp.tile([P, 1], f32, name="warm", tag="warm", bufs=1)
            nc.vector.memset(warm[:], 0.0)
            nc.scalar.activation(warm[:], warm[:], AF.Derivative_Erf)

            # Warm the PE HAM clock gate: junk matmuls while the real
            # inputs are still loading, so real matmuls run at 2.4GHz.
            junk = constp.tile([P, 512], f16)
            nc.gpsimd.memset(junk[:], 0.0)
            for _ in range(6):
                jp = ps.tile([P, 512], f32, name="jp", tag="pt")
                nc.tensor.matmul(jp[:], junk[:, 0:P], junk[:],
                                 start=True, stop=True)

            # meta (bias + rcol, both batches) in ONE DMA: packing gives
            # 1152B-per-partition descriptors instead of 16B ones, so it
            # lands ~3us earlier and unblocks the first score op.
            meta = constp.tile([P, BPC, MW], f32)
            nc.sync.dma_start(
                out=meta[:], in_=meta_d.rearrange("p (b w) -> p b w", b=BPC)
            )

            # trow (frame indices 1..T) from GpSimd iota, int16 (half the
            # bytes of f32; ACT consumes int16 directly). Piece boundaries
            # track the score ops' needs so scores never stall on iota.
            trow = constp.tile([P, T], i16)
            iota_cuts = sorted({0, 512, 1024, T} | {
                min(band[1] + P, T) // P * P
                for bands in bands2 for band in bands if band
            })
            for q0, q1 in zip(iota_cuts, iota_cuts[1:]):
                nc.gpsimd.iota(trow[:, q0:q1],
                               pattern=[[1, q1 - q0]], base=1 + q0,
                               channel_multiplier=0)

            # xs input DMAs up front on the Sync queue, before any output
            # issue can block them (the queue drains in program order).
            tiles = []
            for b in range(BPC):
                bcol = meta[:, b, 0:KC]
                rcolt = meta[:, b, KC:MW]
                xs = sb.tile([P, KC, D], f16, name="xs_t", tag="xs_t")
                xs_src = xs_d[b].rearrange("(k p) d -> p k d", p=P)
                for k in range(KC):
                    nc.sync.dma_start(
                        out=xs[:, k:k + 1, :], in_=xs_src[:, k:k + 1, :]
                    )
                tiles.append((bcol, rcolt, xs))

            score_tiles = [
                sb.tile([P, KC, T], f16, name="scores", tag="scores")
                for _ in range(BPC)
            ]

            # Output-group order: batch 0 leads while batch 1's scores are
            # still being produced, then the two batches interleave so the
            # engines see no cliff at the batch transition. Final groups
            # are split small to shorten the drain tail.
            def batch_groups():
                gs = [list(range(g * OG, (g + 1) * OG))
                      for g in range(MC // OG)]
                return gs[:-1] + [gs[-1][0:2], gs[-1][2:4]]

            g0, g1 = batch_groups(), batch_groups()
            group_seq = [(0, g0[i]) for i in range(3)]
            for i in range(5):
                group_seq.append((0, g0[3 + i]))
                group_seq.append((1, g1[i]))
            group_seq.append((0, g0[8]))
            for i in range(4):
                group_seq.append((1, g1[5 + i]))

            # Score pieces are emitted JUST IN TIME into the ACT stream:
            # each piece goes right before the first group whose matmuls
            # read it, so ACT interleaves score production with its share
            # of normalizes and neither PE nor DVE ever starves on it.
            due = {i: [] for i in range(len(group_seq))}
            for b in range(BPC):
                first_seq = {}
                for seq_idx, (bb, ms) in enumerate(group_seq):
                    if bb != b:
                        continue
                    for m in ms:
                        first_seq.setdefault(m, seq_idx)
                for k, t0, t1 in pieces2[b]:
                    need = min(first_seq[m] for m in range(t0 // P, t1 // P))
                    due[need].append((b, k, t0, t1))

            # Greedy normalize-engine assignment against a simple cost
            # model, accounting for the score pieces on the ACT stream.
            act_t, dve_t = 2000.0, 0.0     # table-load head start
            for seq_idx, (b, ms) in enumerate(group_seq):
                for bb, k, t0, t1 in due[seq_idx]:
                    bcol, rcolt, xs = tiles[bb]
                    nc.scalar.activation(
                        score_tiles[bb][:, k, t0:t1], trow[:, t0:t1],
                        AF.Derivative_Erf, bias=bcol[:, k:k + 1], scale=RSV,
                    )
                    act_t += ACT_FIX + (t1 - t0) * ACT_RATE
                bcol, rcolt, xs = tiles[b]
                scores = score_tiles[b]
                ot = outp.tile([P, len(ms), D], i8, name="ot", tag="ot")
                for g, m in enumerate(ms):
                    ks = mk2[b][m]
                    pt = ps.tile([P, D], f32, name="pt", tag="pt")
                    for i, k in enumerate(ks):
                        lhsT = scores[:, k, m * P:(m + 1) * P]
                        st = (i == 0)
                        sp = (i == len(ks) - 1)
                        mma = nc.tensor.matmul(
                            pt[:, 0:N0], lhsT, xs[:, k, 0:N0],
                            start=st, stop=sp,
                        )
                        mmb = nc.tensor.matmul(
                            pt[:, N0:D], lhsT, xs[:, k, N0:D],
                            start=st, stop=sp,
                        )
                        add_dep_helper(mmb.ins, mma.ins,
                                       reason="keep N-pieces adjacent")
                    if act_t + ACT_COPY <= dve_t + DVE_COPY:
                        act_t += ACT_COPY
                        nc.scalar.activation(
                            ot[:, g, :], pt[:], AF.Copy,
                            scale=rcolt[:, m:m + 1],
                        )
                    else:
                        dve_t += DVE_COPY
                        nc.vector.tensor_scalar_mul(
                            ot[:, g, :], pt[:], rcolt[:, m:m + 1]
                        )
                nc.sync.dma_start(
                    out=out_d[b, ms[0] * P:(ms[-1] + 1) * P, :]
                    .rearrange("(g p) d -> p g d", p=P),
                    in_=ot[:],
                )

    nc.compile()
    return nc


def _get_program(bands):
    prog = _PROGRAMS.get(bands)
    if prog is None:
        prog = _build_program(bands)
        _PROGRAMS[bands] = prog
    return prog


def _denominators(c, mask):
    """Banded softmax denominators: den[b, t-1] = sum_s exp(-(t-c_s)^2/10)
    over valid s, float64, windowed to |t-c| <= DENOM_WIN (terms beyond
    are < 1e-50: irrelevant at fp32)."""
    den = np.zeros((B, T), dtype=np.float64)
    t = np.arange(1, T + 1, dtype=np.float64)
    for b in range(B):
        cb = c[b][mask[b]]
        lo = np.searchsorted(cb, t - DENOM_WIN)
        hi = np.searchsorted(cb, t + DENOM_WIN)
        w = int(np.max(hi - lo)) if len(cb) else 0
        if w == 0:
            continue
        idx = lo[:, None] + np.arange(w)[None, :]
        valid = idx < hi[:, None]
        idx = np.minimum(idx, len(cb) - 1)
        z = t[:, None] - cb[idx]
        terms = np.exp(-(z * z) / VARIANCE) * valid
        den[b] = terms.sum(axis=1)
    return den


def _assign_chunks(cb):
    """Assign sorted centers to KC chunks of capacity P by frame window
    (chunk k targets centers < (k+1)*WIN), greedy forward spill on
    overflow. Returns per-chunk lists of token indices into cb."""
    chunks = [[] for _ in range(KC)]
    k = 0
    for j, cv in enumerate(cb):
        while k < KC - 1 and (cv >= (k + 1) * WIN or len(chunks[k]) >= P):
            k += 1
        kk = k
        while len(chunks[kk]) >= P:
            kk += 1
        chunks[kk].append(j)
    return chunks


def _prepare(x, d, mask):
    x = np.asarray(x, dtype=np.float32)
    d64 = np.asarray(d, dtype=np.float64)
    mask = np.asarray(mask, dtype=bool)

    e = np.cumsum(d64, axis=-1)
    c = e - 0.5 * d64                      # (B, S) token centers

    # Window re-chunking: gather tokens into center-value chunks.
    scale = QMAX / max(float(np.abs(x).max()), 1e-30)
    xs_all = np.zeros((B, S, D), dtype=np.float16)
    cg = np.full((B, S), np.nan)           # gathered centers (nan = pad)
    for b in range(B):
        valid = np.nonzero(mask[b])[0]     # ascending position = ascending c
        cb = c[b][valid]
        for k, idxs in enumerate(_assign_chunks(cb)):
            if not idxs:
                continue
            src = valid[idxs]
            dst = slice(k * P, k * P + len(idxs))
            xs_all[b, dst] = (x[b, src] * scale).astype(np.float16)
            cg[b, dst] = c[b, src]

    # Sort batches by valid length into per-core slots (similar lengths
    # share a slot so the per-slot band unions stay tight).
    order = np.argsort(mask.sum(1), kind="stable")
    bands2 = tuple(
        _compute_bands(cg[order[s * NCORES:(s + 1) * NCORES]])
        for s in range(BPC)
    )

    cbias = np.where(np.isnan(cg), -1.0e4, cg)   # pad tokens: derf gives 0
    bias = (-cbias * RSV).astype(np.float32)

    den = _denominators(c, mask)           # (B, T) float64
    rcol = (1.0 / (C_DE * den)).astype(np.float32)    # (B, T)
    rcol = rcol.reshape(B, MC, P).transpose(0, 2, 1)  # (B, P, MC)

    MW = KC + MC
    in_maps = []
    for core in range(NCORES):
        idx = [order[core], order[NCORES + core]]
        meta = np.empty((P, BPC, MW), dtype=np.float32)
        for b, bi in enumerate(idx):
            meta[:, b, 0:KC] = bias[bi].reshape(KC, P).T
            meta[:, b, KC:MW] = rcol[bi]
        in_maps.append({
            "xs": np.ascontiguousarray(xs_all[idx]),
            "meta": meta.reshape(P, BPC * MW).copy(),
        })
    return in_maps, bands2, order, scale


def run(x, d, mask, frame_length, trace=False):
    assert int(frame_length) == T
    in_maps, bands2, order, scale = _prepare(x, d, mask)
    nc = _get_program(bands2)
    res = None
    for attempt in range(3):
        try:
            res = run_bass_kernel_spmd(nc, in_maps, list(range(NCORES)),
                                       trace=trace)
            break
        except Exception:
            # The first execution after a fresh compile occasionally hits a
            # transient device error; retrying succeeds.
            if attempt == 2:
                raise
    inv = np.float32(1.0 / scale)
    out = np.empty((B, T, D), dtype=np.float32)
    for core in range(NCORES):
        for s in range(BPC):
            q = res.results[core]["out"][s]
            out[order[s * NCORES + core]] = q.astype(np.float32) * inv
    return out, res


def kernel(x, d, mask, frame_length):
    out, _ = run(x, d, mask, frame_length, trace=False)
    return out
